# revision 14
# baseline (speedup 1.0000x reference)
"""Trainium2 Bass kernel for a 5-layer GRU encoder (bi-GRU + 3 stacked GRUs).

Layout convention ("transposed"/unit-major):
  - state tiles:  [128 part = unit-sub s, free = 32*q + b]  (q = unit chunk 0..3, b = batch 0..31)
  - PSUM rec:     [128 part = s, free = 128*g + 32*q + b]   (g = gate z/r/h)
  - xp (X) tiles: same free layout as PSUM rec, per timestep, streamed from DRAM
  - token order:  tok = 32*t + b  (time-major)

Per GRU step (Keras reset_after=True, sigmoid activations everywhere):
  z = sig(xz + rz);  r = sig(xr + rr);  hh = sig(xh + r*(rec_h + brec_h))
  hn = z*h + (1-z)*hh ; masked steps carry h.
We pre-negate the z columns of W/U/biases on the host so PSUM holds -(xz+rz)
and one sigmoid yields z' = 1-z; the pad mask is folded into X's z region as
-1e9 so z'=0 (carry) at masked steps:  hn = h + z'*(hh - h).
"""

import numpy as np

VOCAB = 32000
EMB = 512
UNITS = 512
B = 32
T = 128
NCORES = 8

F32 = None  # set lazily (mybir dtypes)
BF16 = None

_CACHE = {}


def _dt():
    import concourse.mybir as mybir

    return mybir.dt.float32, mybir.dt.bfloat16, mybir.dt.int16


# ---------------------------------------------------------------------------
# host-side weight packing
# ---------------------------------------------------------------------------

def _pack_stationary(W, negate_z):
    """W: [din, 3U] -> stationary pack [128, nk*12*128] bf16.

    col ((kd*12 + m)*128 + s), part kp  =  W[128*kd + kp, 512*g + 128*q + s]
    with m = 4*g + q.  z columns (g=0) negated when negate_z.
    """
    din = W.shape[0]
    nk = din // 128
    W = W.astype(np.float32).copy()
    if negate_z:
        W[:, 0:UNITS] = -W[:, 0:UNITS]
    # [din, 1536] -> [nk, 128(kp), g, q, s=128]
    Wr = W.reshape(nk, 128, 3, 4, 128)
    # -> [kp, nk, g*4+q, s]
    Wp = Wr.transpose(1, 0, 2, 3, 4).reshape(128, nk * 12 * 128)
    return _to_bf16(Wp)


def _to_bf16(a):
    import ml_dtypes

    return np.asarray(a, dtype=np.float32).astype(ml_dtypes.bfloat16)


def _gate_bias(b, negate_z):
    """b: [2, 3U] -> (bias_zr_pack [128,12] f32 (z,r: b_in+b_rec ; h: b_in),
                      brh [128,128] bf16 (b_rec_h broadcast))"""
    b_in, b_rec = b[0].astype(np.float32), b[1].astype(np.float32)
    tot = b_in + b_rec            # for z, r
    bias = np.zeros((128, 12), np.float32)
    for g in range(3):
        src = tot if g < 2 else b_in
        for q in range(4):
            col = src[512 * g + 128 * q: 512 * g + 128 * (q + 1)].copy()
            if g == 0 and negate_z:
                col = -col
            bias[:, 4 * g + q] = col
    brh = np.zeros((128, 128), np.float32)
    for q in range(4):
        brh[:, 32 * q: 32 * (q + 1)] = b_rec[1024 + 128 * q: 1024 + 128 * (q + 1)][:, None]
    return bias, _to_bf16(brh)


def prep_inputs(inputs, nT=T):
    """Host-side prep: cast/pack weights, build indices and mask."""
    x = np.asarray(inputs["x"]).astype(np.int64)          # [B, T]
    E = np.asarray(inputs["E"], np.float32)

    ins = {}
    ins["E16"] = _to_bf16(E)

    # token order tok = 32*t + b ; wrapped idx layout [16, ntok/16]
    xt = x.T.reshape(-1)[: B * nT]                        # tok = t*B + b
    idx = np.zeros((128, (B * nT) // 16), np.int16)
    for i in range(B * nT):
        idx[i % 16, i // 16] = np.int16(xt[i])
    ins["xidx"] = idx

    maskneg = np.where(xt == 0, np.float32(-1e9), np.float32(0.0))
    ins["maskneg"] = _to_bf16(np.broadcast_to(maskneg[None, :], (128, B * nT)).copy())

    for name in ("fw", "bw", "m0", "m1", "top"):
        ins[f"W_{name}p"] = _pack_stationary(np.asarray(inputs[f"W_{name}"]), True)
        ins[f"U_{name}p"] = _pack_stationary(np.asarray(inputs[f"U_{name}"]), True)
        bias, brh = _gate_bias(np.asarray(inputs[f"b_{name}"]), True)
        ins[f"bias_{name}"] = bias
        ins[f"brh_{name}"] = brh

    ins["ident16"] = _to_bf16(np.eye(128, dtype=np.float32))
    ins["ident32"] = np.eye(128, dtype=np.float32)
    return ins


# ---------------------------------------------------------------------------
# device program
# ---------------------------------------------------------------------------

def _bulk_xp(nc, tc, pools, Wt, movings, Xd, bias_sb, maskneg_sb, nT, sig):
    """xp for one layer: X[t,s,c] = sum_k in^T @ W  (+bias, +mask on z).

    Wt: stationary pack tile [128, nk*12*128] bf16
    movings: list of nk APs, each [128, ntok] bf16 (k-chunk of input^T)
    Xd: DRAM [nT, 128, 384] bf16
    bias_sb: [128, 12] f32 AP  (per (g,q) block, z pre-negated)
    """
    import concourse.mybir as mybir

    f32, bf16, _ = _dt()
    nk, moving_fn = movings
    tch = min(16, nT)              # timesteps per 512-token chunk
    nch = nT // tch
    csz = tch * B
    for m in range(12):
        for c in range(nch):
            P = pools["bulkp"].tile([128, csz], f32, tag="bulkP", name="bulkP")
            for k in range(nk):
                nc.tensor.matmul(
                    P[:, :],
                    Wt[:, (k * 12 + m) * 128: (k * 12 + m + 1) * 128],
                    moving_fn(k, c),
                    start=(k == 0),
                    stop=(k == nk - 1),
                )
            S = pools["bulks"].tile([128, csz], bf16, tag="bulkS", name="bulkS")
            if m < 4:  # z block: bias + mask fold, one DVE op
                nc.vector.scalar_tensor_tensor(
                    S[:, :], P[:, :], bias_sb[:, m: m + 1],
                    maskneg_sb[:, c * csz: (c + 1) * csz],
                    mybir.AluOpType.add, mybir.AluOpType.add,
                )
            else:
                nc.scalar.activation(
                    S[:, :], P[:, :], mybir.ActivationFunctionType.Identity,
                    bias=bias_sb[:, m: m + 1],
                )
            # X[t, s, 32*m : 32*m+32] for the tch timesteps in this chunk
            dst = Xd[c * tch: (c + 1) * tch, :, 32 * m: 32 * (m + 1)]
            nc.sync.dma_start(dst.rearrange("t p b -> p t b"),
                              S[:, :].rearrange("p (t b) -> p t b", b=B))


def _scan(nc, tc, pools, Ut, Xd, Y, brh_sb, ident16, nT, reverse, sig,
          y_out=None, hT_out=None, ident32=None):
    """One GRU scan. Y: SBUF tile [128, nT*128] bf16 output (also next-step state).
    If y_out is given (top layer), also emit transposed per-step outputs."""
    import concourse.mybir as mybir

    f32, bf16, _ = _dt()
    Sig = mybir.ActivationFunctionType.Sigmoid

    h_prev = pools["state"].tile([128, 128], f32, tag=f"h{sig}", name=f"h{sig}")
    nc.vector.memset(h_prev[:, :], 0.0)

    order = range(nT - 1, -1, -1) if reverse else range(nT)
    first = True
    for t in order:
        Xt = pools["xstream"].tile([128, 384], bf16, tag="Xt", name="Xt")
        nc.sync.dma_start(Xt[:, :], Xd[t])
        R = pools["scanp"].tile([128, 384], f32, tag="R", name="R")
        # inject all of xp into PSUM via identity matmul (single start per bank)
        nc.tensor.matmul(R[:, 0:384], ident16[:, :], Xt[:, 0:384],
                         start=True, stop=first)
        if not first:
            for kq in range(4):
                rhs = Y[:, 128 * tprev + 32 * kq: 128 * tprev + 32 * (kq + 1)]
                for m in range(12):
                    nc.tensor.matmul(
                        R[:, 32 * m: 32 * (m + 1)],
                        Ut[:, (kq * 12 + m) * 128: (kq * 12 + m + 1) * 128],
                        rhs,
                        start=False,
                        stop=(kq == 3 and m == 11),
                    )
        # gates: PSUM holds [-(xz+rz) | xr+rr | xh+rec_h]
        zr = pools["gates"].tile([128, 256], f32, tag="zr", name="zr")
        nc.scalar.activation(zr[:, :], R[:, 0:256], Sig)
        a = pools["gates"].tile([128, 128], f32, tag="a", name="a")
        nc.vector.tensor_sub(a[:, :], R[:, 256:384], Xt[:, 256:384])
        if brh_sb is not None:
            a2 = pools["gates"].tile([128, 128], f32, tag="a2", name="a2")
            nc.vector.tensor_add(a2[:, :], a[:, :], brh_sb[:, :])
            a = a2
        t1 = pools["gates"].tile([128, 128], f32, tag="t1", name="t1")
        nc.vector.tensor_mul(t1[:, :], zr[:, 128:256], a[:, :])
        hp = pools["gates"].tile([128, 128], f32, tag="hp", name="hp")
        nc.vector.tensor_add(hp[:, :], t1[:, :], Xt[:, 256:384])
        hh = pools["gates"].tile([128, 128], f32, tag="hh", name="hh")
        nc.scalar.activation(hh[:, :], hp[:, :], Sig)
        d = pools["gates"].tile([128, 128], f32, tag="d", name="d")
        nc.vector.tensor_sub(d[:, :], hh[:, :], h_prev[:, :])
        wd = pools["gates"].tile([128, 128], f32, tag="wd", name="wd")
        nc.vector.tensor_mul(wd[:, :], zr[:, 0:128], d[:, :])
        h_new = pools["state"].tile([128, 128], f32, tag=f"h{sig}", name=f"h{sig}")
        nc.vector.tensor_add(h_new[:, :], h_prev[:, :], wd[:, :])
        nc.vector.tensor_copy(Y[:, 128 * t: 128 * (t + 1)], h_new[:, :])

        if y_out is not None:
            Py = pools["ypsum"].tile([32, 512], f32, tag="Py", name="Py")
            for q in range(4):
                nc.tensor.transpose(Py[:, 128 * q: 128 * (q + 1)],
                                    h_new[:, 32 * q: 32 * (q + 1)],
                                    ident32[:, :])
            ys = pools["ystage"].tile([32, 512], f32, tag="ys", name="ys")
            nc.scalar.activation(ys[:, :], Py[:, :],
                                 mybir.ActivationFunctionType.Copy)
            nc.sync.dma_start(y_out[:, t, :], ys[:, :])
            if hT_out is not None and t == nT - 1:
                nc.sync.dma_start(hT_out[:, :], ys[:, :])

        h_prev = h_new
        tprev = t
        first = False


def build(nT=T, has_brh=()):
    import concourse.bacc as bacc
    import concourse.mybir as mybir
    import concourse.tile as tile

    f32, bf16, i16 = _dt()
    ntok = B * nT

    nc = bacc.Bacc(None, target_bir_lowering=False,
                   dynamic_dma_scratch_size=2 ** 16)

    # I/O
    E16 = nc.dram_tensor("E16", [VOCAB, EMB], bf16, kind="ExternalInput")
    xidx = nc.dram_tensor("xidx", [128, ntok // 16], i16, kind="ExternalInput")
    maskneg = nc.dram_tensor("maskneg", [128, ntok], bf16, kind="ExternalInput")
    Wp = {}
    for name, nk in (("fw", 4), ("bw", 4), ("m0", 8), ("m1", 4), ("top", 4)):
        Wp[name] = nc.dram_tensor(f"W_{name}p", [128, nk * 12 * 128], bf16,
                                  kind="ExternalInput")
    Up = {name: nc.dram_tensor(f"U_{name}p", [128, 4 * 12 * 128], bf16,
                               kind="ExternalInput")
          for name in ("fw", "bw", "m0", "m1", "top")}
    biasd = {name: nc.dram_tensor(f"bias_{name}", [128, 12], f32,
                                  kind="ExternalInput")
             for name in ("fw", "bw", "m0", "m1", "top")}
    brhd = {name: nc.dram_tensor(f"brh_{name}", [128, 128], bf16,
                                 kind="ExternalInput")
            for name in ("fw", "bw", "m0", "m1", "top")}
    ident16_d = nc.dram_tensor("ident16", [128, 128], bf16, kind="ExternalInput")
    ident32_d = nc.dram_tensor("ident32", [128, 128], f32, kind="ExternalInput")

    y_out = nc.dram_tensor("y", [B, nT, UNITS], f32, kind="ExternalOutput")
    hT_out = nc.dram_tensor("hT", [B, UNITS], f32, kind="ExternalOutput")

    # internal DRAM xp buffers
    Xd = {name: nc.dram_tensor(f"X_{name}", [nT, 128, 384], bf16)
          for name in ("fw", "bw", "m0", "m1", "top")}

    with tile.TileContext(nc) as tc:
        import contextlib

        with contextlib.ExitStack() as stk:
            const = stk.enter_context(tc.tile_pool(name="const", bufs=1))
            wpool = stk.enter_context(tc.tile_pool(name="wpool", bufs=2))
            pools = {
                "bulkp": stk.enter_context(
                    tc.tile_pool(name="bulkp", bufs=3, space="PSUM")),
                "bulks": stk.enter_context(tc.tile_pool(name="bulks", bufs=3)),
                "scanp": stk.enter_context(
                    tc.tile_pool(name="scanp", bufs=2, space="PSUM")),
                "xstream": stk.enter_context(tc.tile_pool(name="xstream", bufs=3)),
                "gates": stk.enter_context(tc.tile_pool(name="gates", bufs=2)),
                "state": stk.enter_context(tc.tile_pool(name="state", bufs=2)),
                "ypsum": stk.enter_context(
                    tc.tile_pool(name="ypsum", bufs=2, space="PSUM")),
                "ystage": stk.enter_context(tc.tile_pool(name="ystage", bufs=2)),
            }

            ident16 = const.tile([128, 128], bf16)
            nc.sync.dma_start(ident16[:, :], ident16_d[:, :])
            ident32 = const.tile([128, 128], f32)
            nc.sync.dma_start(ident32[:, :], ident32_d[:, :])
            mask_sb = const.tile([128, ntok], bf16)
            nc.sync.dma_start(mask_sb[:, :], maskneg[:, :])
            bias_sb = {}
            brh_sb = {}
            for name in ("fw", "bw", "m0", "m1", "top"):
                bias_sb[name] = const.tile([128, 12], f32, tag=f"bias_{name}", name=f"biassb_{name}")
                nc.sync.dma_start(bias_sb[name][:, :], biasd[name][:, :])
                if name in has_brh:
                    brh_sb[name] = const.tile([128, 128], bf16, tag=f"brh_{name}", name=f"brhsb_{name}")
                    nc.sync.dma_start(brh_sb[name][:, :], brhd[name][:, :])
                else:
                    brh_sb[name] = None

            idx_sb = const.tile([128, ntok // 16], i16)
            nc.sync.dma_start(idx_sb[:, :], xidx[:, :])

            def load_w(dram, nk):
                t_ = wpool.tile([128, nk * 12 * 128], bf16, tag="W", name="Wt")
                nc.sync.dma_start(t_[:, :], dram[:, :])
                return t_

            tch = min(16, nT)

            def y_moving(Ytile, koff=0):
                r = Ytile[:, :].rearrange("p (t q b) -> p q t b", q=4, b=B)
                return lambda k, c: r[:, k - koff, c * tch: (c + 1) * tch, :]

            with tc.tile_pool(name="p01", bufs=1) as p01:
                eT = p01.tile([128, 4 * ntok], bf16)
                gch = min(512, ntok)   # tokens per gather call = bulk chunk
                eTr = eT[:, :].rearrange("p (c k n) -> p c k n", k=4, n=gch)
                for gc in range(ntok // gch):
                    nc.gpsimd.dma_gather(
                        eTr[:, gc],
                        E16[:, :],
                        idx_sb[:, gc * (gch // 16): (gc + 1) * (gch // 16)],
                        num_idxs=gch,
                        num_idxs_reg=gch,
                        elem_size=EMB,
                        transpose=True,
                    )
                e_moving = (4, lambda k, c: eTr[:, c, k, :])

                Wt = load_w(Wp["fw"], 4)
                _bulk_xp(nc, tc, pools, Wt, e_moving, Xd["fw"],
                         bias_sb["fw"][:, :], mask_sb[:, :], nT, "xf")
                Wt = load_w(Wp["bw"], 4)
                _bulk_xp(nc, tc, pools, Wt, e_moving, Xd["bw"],
                         bias_sb["bw"][:, :], mask_sb[:, :], nT, "xb")

            with tc.tile_pool(name="p23", bufs=1) as p23:
                Yfw = p23.tile([128, nT * 128], bf16, tag="Yfw", name="Yfw")
                Ybw = p23.tile([128, nT * 128], bf16, tag="Ybw", name="Ybw")
                Ufw_t = load_w(Up["fw"], 4)
                Ubw_t = load_w(Up["bw"], 4)
                _scan(nc, tc, pools, Ufw_t, Xd["fw"], Yfw, brh_sb["fw"],
                      ident16, nT, False, "fw")
                _scan(nc, tc, pools, Ubw_t, Xd["bw"], Ybw, brh_sb["bw"],
                      ident16, nT, True, "bw")

                Wt = load_w(Wp["m0"], 8)
                mf, mb = y_moving(Yfw), y_moving(Ybw, 4)
                _bulk_xp(nc, tc, pools, Wt,
                         (8, lambda k, c: mf(k, c) if k < 4 else mb(k, c)),
                         Xd["m0"], bias_sb["m0"][:, :], mask_sb[:, :], nT, "x0")

            with tc.tile_pool(name="p45", bufs=1) as p45:
                Ym0 = p45.tile([128, nT * 128], bf16, tag="Ym0", name="Ym0")
                Ut = load_w(Up["m0"], 4)
                _scan(nc, tc, pools, Ut, Xd["m0"], Ym0, brh_sb["m0"],
                      ident16, nT, False, "m0")
                Wt = load_w(Wp["m1"], 4)
                _bulk_xp(nc, tc, pools, Wt, (4, y_moving(Ym0)), Xd["m1"],
                         bias_sb["m1"][:, :], mask_sb[:, :], nT, "x1")

            with tc.tile_pool(name="p67", bufs=1) as p67:
                Ym1 = p67.tile([128, nT * 128], bf16, tag="Ym1", name="Ym1")
                Ut = load_w(Up["m1"], 4)
                _scan(nc, tc, pools, Ut, Xd["m1"], Ym1, brh_sb["m1"],
                      ident16, nT, False, "m1")
                Wt = load_w(Wp["top"], 4)
                _bulk_xp(nc, tc, pools, Wt, (4, y_moving(Ym1)), Xd["top"],
                         bias_sb["top"][:, :], mask_sb[:, :], nT, "xt")

            with tc.tile_pool(name="p89", bufs=1) as p89:
                Ytop = p89.tile([128, nT * 128], bf16, tag="Ytop", name="Ytop")
                Ut = load_w(Up["top"], 4)
                _scan(nc, tc, pools, Ut, Xd["top"], Ytop, brh_sb["top"],
                      ident16, nT, False, "top",
                      y_out=y_out, hT_out=hT_out, ident32=ident32)

    nc.finalize()
    return nc


RUN_CORES = 1


def _run_hw(nc, ins):
    from concourse.bass_utils import run_bass_kernel_spmd

    in_maps = [ins for _ in range(RUN_CORES)]
    res = run_bass_kernel_spmd(nc, in_maps, list(range(RUN_CORES)))
    return res.results[0]


def kernel(**inputs):
    nT = T
    has_brh = tuple(sorted(
        name for name in ("fw", "bw", "m0", "m1", "top")
        if np.any(np.asarray(inputs[f"b_{name}"])[1, 1024:] != 0)
    ))
    key = ("nc", nT, has_brh)
    if key not in _CACHE:
        _CACHE[key] = build(nT, has_brh)
    nc = _CACHE[key]
    ins = prep_inputs(inputs, nT)
    out = _run_hw(nc, ins)
    return np.asarray(out["y"], np.float32), np.asarray(out["hT"], np.float32)


# revision 17
# speedup vs baseline: 1.0066x; 1.0066x over previous
"""Trainium2 Bass kernel for a 5-layer GRU encoder (bi-GRU + 3 stacked GRUs).

Layout convention ("transposed"/unit-major):
  - state tiles:  [128 part = unit-sub s, free = 32*q + b]  (q = unit chunk 0..3, b = batch 0..31)
  - PSUM rec:     [128 part = s, free = 128*g + 32*q + b]   (g = gate z/r/h)
  - xp (X) tiles: same free layout as PSUM rec, per timestep, streamed from DRAM
  - token order:  tok = 32*t + b  (time-major)

Per GRU step (Keras reset_after=True, sigmoid activations everywhere):
  z = sig(xz + rz);  r = sig(xr + rr);  hh = sig(xh + r*(rec_h + brec_h))
  hn = z*h + (1-z)*hh ; masked steps carry h.
We pre-negate the z columns of W/U/biases on the host so PSUM holds -(xz+rz)
and one sigmoid yields z' = 1-z; the pad mask is folded into X's z region as
-1e9 so z'=0 (carry) at masked steps:  hn = h + z'*(hh - h).
"""

import numpy as np

VOCAB = 32000
EMB = 512
UNITS = 512
B = 32
T = 128
NCORES = 8

F32 = None  # set lazily (mybir dtypes)
BF16 = None

_CACHE = {}


def _dt():
    import concourse.mybir as mybir

    return mybir.dt.float32, mybir.dt.bfloat16, mybir.dt.int16


# ---------------------------------------------------------------------------
# host-side weight packing
# ---------------------------------------------------------------------------

def _pack_stationary(W, negate_z):
    """W: [din, 3U] -> stationary pack [128, nk*12*128] bf16.

    col ((kd*12 + m)*128 + s), part kp  =  W[128*kd + kp, 512*g + 128*q + s]
    with m = 4*g + q.  z columns (g=0) negated when negate_z.
    """
    din = W.shape[0]
    nk = din // 128
    W = W.astype(np.float32).copy()
    if negate_z:
        W[:, 0:UNITS] = -W[:, 0:UNITS]
    # [din, 1536] -> [nk, 128(kp), g, q, s=128]
    Wr = W.reshape(nk, 128, 3, 4, 128)
    # -> [kp, nk, g*4+q, s]
    Wp = Wr.transpose(1, 0, 2, 3, 4).reshape(128, nk * 12 * 128)
    return _to_bf16(Wp)


def _to_bf16(a):
    import ml_dtypes

    return np.asarray(a, dtype=np.float32).astype(ml_dtypes.bfloat16)


def _gate_bias(b, negate_z):
    """b: [2, 3U] -> (bias_zr_pack [128,12] f32 (z,r: b_in+b_rec ; h: b_in),
                      brh [128,128] bf16 (b_rec_h broadcast))"""
    b_in, b_rec = b[0].astype(np.float32), b[1].astype(np.float32)
    tot = b_in + b_rec            # for z, r
    bias = np.zeros((128, 12), np.float32)
    for g in range(3):
        src = tot if g < 2 else b_in
        for q in range(4):
            col = src[512 * g + 128 * q: 512 * g + 128 * (q + 1)].copy()
            if g == 0 and negate_z:
                col = -col
            bias[:, 4 * g + q] = col
    brh = np.zeros((128, 128), np.float32)
    for q in range(4):
        brh[:, 32 * q: 32 * (q + 1)] = b_rec[1024 + 128 * q: 1024 + 128 * (q + 1)][:, None]
    return bias, _to_bf16(brh)


def prep_inputs(inputs, nT=T):
    """Host-side prep: cast/pack weights, build indices and mask."""
    x = np.asarray(inputs["x"]).astype(np.int64)          # [B, T]
    E = np.asarray(inputs["E"], np.float32)

    ins = {}
    ins["E16"] = _to_bf16(E)

    # token order tok = 32*t + b ; wrapped idx layout [16, ntok/16]
    xt = x.T.reshape(-1)[: B * nT]                        # tok = t*B + b
    idx = np.zeros((128, (B * nT) // 16), np.int16)
    for i in range(B * nT):
        idx[i % 16, i // 16] = np.int16(xt[i])
    ins["xidx"] = idx

    maskneg = np.where(xt == 0, np.float32(-1e9), np.float32(0.0))
    ins["maskneg"] = _to_bf16(np.broadcast_to(maskneg[None, :], (128, B * nT)).copy())

    for name in ("fw", "bw", "m0", "m1", "top"):
        ins[f"W_{name}p"] = _pack_stationary(np.asarray(inputs[f"W_{name}"]), True)
        ins[f"U_{name}p"] = _pack_stationary(np.asarray(inputs[f"U_{name}"]), True)
        bias, brh = _gate_bias(np.asarray(inputs[f"b_{name}"]), True)
        ins[f"bias_{name}"] = bias
        ins[f"brh_{name}"] = brh

    ins["ident16"] = _to_bf16(np.eye(128, dtype=np.float32))
    ins["ident32"] = np.eye(128, dtype=np.float32)
    return ins


# ---------------------------------------------------------------------------
# device program
# ---------------------------------------------------------------------------

def _bulk_xp(nc, tc, pools, Wt, movings, Xd, bias_sb, maskneg_sb, nT, sig):
    """xp for one layer: X[t,s,c] = sum_k in^T @ W  (+bias, +mask on z).

    Wt: stationary pack tile [128, nk*12*128] bf16
    movings: list of nk APs, each [128, ntok] bf16 (k-chunk of input^T)
    Xd: DRAM [nT, 128, 384] bf16
    bias_sb: [128, 12] f32 AP  (per (g,q) block, z pre-negated)
    """
    import concourse.mybir as mybir

    f32, bf16, _ = _dt()
    nk, moving_fn = movings
    tch = min(16, nT)              # timesteps per 512-token chunk
    nch = nT // tch
    csz = tch * B

    def emit_chunk(c):
        for m in range(12):
            P = pools["bulkp"].tile([128, csz], f32, tag="bulkP", name="bulkP")
            for k in range(nk):
                nc.tensor.matmul(
                    P[:, :],
                    Wt[:, (k * 12 + m) * 128: (k * 12 + m + 1) * 128],
                    moving_fn(k, c),
                    start=(k == 0),
                    stop=(k == nk - 1),
                )
            S = pools["bulks"].tile([128, csz], bf16, tag="bulkS", name="bulkS")
            if m < 4:  # z block: bias + mask fold, one DVE op
                nc.vector.scalar_tensor_tensor(
                    S[:, :], P[:, :], bias_sb[:, m: m + 1],
                    maskneg_sb[:, c * csz: (c + 1) * csz],
                    mybir.AluOpType.add, mybir.AluOpType.add,
                )
            else:
                nc.scalar.activation(
                    S[:, :], P[:, :], mybir.ActivationFunctionType.Identity,
                    bias=bias_sb[:, m: m + 1],
                )
            # X[t, s, 32*m : 32*m+32] for the tch timesteps in this chunk
            dst = Xd[c * tch: (c + 1) * tch, :, 32 * m: 32 * (m + 1)]
            nc.sync.dma_start(dst.rearrange("t p b -> p t b"),
                              S[:, :].rearrange("p (t b) -> p t b", b=B))

    return nch, emit_chunk


def _scan(nc, tc, pools, Ut, Xd, Y, brh_sb, ident16, nT, reverse, sig,
          y_out=None, hT_out=None, ident32=None):
    """One GRU scan. Y: SBUF tile [128, nT*128] bf16 output (also next-step state).
    If y_out is given (top layer), also emit transposed per-step outputs."""
    import concourse.mybir as mybir

    f32, bf16, _ = _dt()
    Sig = mybir.ActivationFunctionType.Sigmoid

    h_prev = pools["state"].tile([128, 128], f32, tag=f"h{sig}", name=f"h{sig}")
    nc.vector.memset(h_prev[:, :], 0.0)

    order = list(range(nT - 1, -1, -1) if reverse else range(nT))
    state = {"h_prev": h_prev, "tprev": None, "first": True}

    def emit_step(i):
        t = order[i]
        h_prev = state["h_prev"]
        tprev = state["tprev"]
        first = state["first"]
        Xt = pools["xstream"].tile([128, 384], bf16, tag="Xt", name="Xt")
        nc.sync.dma_start(Xt[:, :], Xd[t])
        R = pools["scanp"].tile([128, 384], f32, tag="R", name="R")
        # inject all of xp into PSUM via identity matmul (single start per bank)
        nc.tensor.matmul(R[:, 0:384], ident16[:, :], Xt[:, 0:384],
                         start=True, stop=first)
        if not first:
            for kq in range(4):
                rhs = Y[:, 128 * tprev + 32 * kq: 128 * tprev + 32 * (kq + 1)]
                for m in range(12):
                    nc.tensor.matmul(
                        R[:, 32 * m: 32 * (m + 1)],
                        Ut[:, (kq * 12 + m) * 128: (kq * 12 + m + 1) * 128],
                        rhs,
                        start=False,
                        stop=(kq == 3 and m == 11),
                    )
        # gates: PSUM holds [-(xz+rz) | xr+rr | xh+rec_h]
        zr = pools["gates"].tile([128, 256], f32, tag="zr", name="zr")
        nc.scalar.activation(zr[:, :], R[:, 0:256], Sig)
        a = pools["gates"].tile([128, 128], f32, tag="a", name="a")
        nc.vector.tensor_sub(a[:, :], R[:, 256:384], Xt[:, 256:384])
        if brh_sb is not None:
            a2 = pools["gates"].tile([128, 128], f32, tag="a2", name="a2")
            nc.vector.tensor_add(a2[:, :], a[:, :], brh_sb[:, :])
            a = a2
        t1 = pools["gates"].tile([128, 128], f32, tag="t1", name="t1")
        nc.vector.tensor_mul(t1[:, :], zr[:, 128:256], a[:, :])
        hp = pools["gates"].tile([128, 128], f32, tag="hp", name="hp")
        nc.vector.tensor_add(hp[:, :], t1[:, :], Xt[:, 256:384])
        hh = pools["gates"].tile([128, 128], f32, tag="hh", name="hh")
        nc.scalar.activation(hh[:, :], hp[:, :], Sig)
        d = pools["gates"].tile([128, 128], f32, tag="d", name="d")
        nc.vector.tensor_sub(d[:, :], hh[:, :], h_prev[:, :])
        wd = pools["gates"].tile([128, 128], f32, tag="wd", name="wd")
        nc.vector.tensor_mul(wd[:, :], zr[:, 0:128], d[:, :])
        h_new = pools["state"].tile([128, 128], f32, tag=f"h{sig}", name=f"h{sig}")
        nc.vector.tensor_add(h_new[:, :], h_prev[:, :], wd[:, :])
        nc.vector.tensor_copy(Y[:, 128 * t: 128 * (t + 1)], h_new[:, :])

        if y_out is not None:
            Py = pools["ypsum"].tile([32, 512], f32, tag="Py", name="Py")
            for q in range(4):
                nc.tensor.transpose(Py[:, 128 * q: 128 * (q + 1)],
                                    h_new[:, 32 * q: 32 * (q + 1)],
                                    ident32[:, :])
            ys = pools["ystage"].tile([32, 512], f32, tag="ys", name="ys")
            nc.scalar.activation(ys[:, :], Py[:, :],
                                 mybir.ActivationFunctionType.Copy)
            nc.sync.dma_start(y_out[:, t, :], ys[:, :])
            if hT_out is not None and t == nT - 1:
                nc.sync.dma_start(hT_out[:, :], ys[:, :])

        state["h_prev"] = h_new
        state["tprev"] = t
        state["first"] = False

    return emit_step


def build(nT=T, has_brh=()):
    import concourse.bacc as bacc
    import concourse.mybir as mybir
    import concourse.tile as tile

    f32, bf16, i16 = _dt()
    ntok = B * nT

    nc = bacc.Bacc(None, target_bir_lowering=False,
                   dynamic_dma_scratch_size=2 ** 16)

    # I/O
    E16 = nc.dram_tensor("E16", [VOCAB, EMB], bf16, kind="ExternalInput")
    xidx = nc.dram_tensor("xidx", [128, ntok // 16], i16, kind="ExternalInput")
    maskneg = nc.dram_tensor("maskneg", [128, ntok], bf16, kind="ExternalInput")
    Wp = {}
    for name, nk in (("fw", 4), ("bw", 4), ("m0", 8), ("m1", 4), ("top", 4)):
        Wp[name] = nc.dram_tensor(f"W_{name}p", [128, nk * 12 * 128], bf16,
                                  kind="ExternalInput")
    Up = {name: nc.dram_tensor(f"U_{name}p", [128, 4 * 12 * 128], bf16,
                               kind="ExternalInput")
          for name in ("fw", "bw", "m0", "m1", "top")}
    biasd = {name: nc.dram_tensor(f"bias_{name}", [128, 12], f32,
                                  kind="ExternalInput")
             for name in ("fw", "bw", "m0", "m1", "top")}
    brhd = {name: nc.dram_tensor(f"brh_{name}", [128, 128], bf16,
                                 kind="ExternalInput")
            for name in ("fw", "bw", "m0", "m1", "top")}
    ident16_d = nc.dram_tensor("ident16", [128, 128], bf16, kind="ExternalInput")
    ident32_d = nc.dram_tensor("ident32", [128, 128], f32, kind="ExternalInput")

    y_out = nc.dram_tensor("y", [B, nT, UNITS], f32, kind="ExternalOutput")
    hT_out = nc.dram_tensor("hT", [B, UNITS], f32, kind="ExternalOutput")

    # internal DRAM xp buffers
    Xd = {name: nc.dram_tensor(f"X_{name}", [nT, 128, 384], bf16)
          for name in ("fw", "bw", "m0", "m1", "top")}

    with tile.TileContext(nc) as tc:
        import contextlib

        with contextlib.ExitStack() as stk:
            const = stk.enter_context(tc.tile_pool(name="const", bufs=1))
            wpool = stk.enter_context(tc.tile_pool(name="wpool", bufs=2))
            wpool8 = stk.enter_context(tc.tile_pool(name="wpool8", bufs=1))
            pools = {
                "bulkp": stk.enter_context(
                    tc.tile_pool(name="bulkp", bufs=3, space="PSUM")),
                "bulks": stk.enter_context(tc.tile_pool(name="bulks", bufs=3)),
                "scanp": stk.enter_context(
                    tc.tile_pool(name="scanp", bufs=2, space="PSUM")),
                "xstream": stk.enter_context(tc.tile_pool(name="xstream", bufs=3)),
                "gates": stk.enter_context(tc.tile_pool(name="gates", bufs=2)),
                "state": stk.enter_context(tc.tile_pool(name="state", bufs=2)),
                "ypsum": stk.enter_context(
                    tc.tile_pool(name="ypsum", bufs=2, space="PSUM")),
                "ystage": stk.enter_context(tc.tile_pool(name="ystage", bufs=2)),
            }

            ident16 = const.tile([128, 128], bf16)
            nc.sync.dma_start(ident16[:, :], ident16_d[:, :])
            ident32 = const.tile([128, 128], f32)
            nc.sync.dma_start(ident32[:, :], ident32_d[:, :])
            mask_sb = const.tile([128, ntok], bf16)
            nc.sync.dma_start(mask_sb[:, :], maskneg[:, :])
            bias_sb = {}
            brh_sb = {}
            for name in ("fw", "bw", "m0", "m1", "top"):
                bias_sb[name] = const.tile([128, 12], f32, tag=f"bias_{name}", name=f"biassb_{name}")
                nc.sync.dma_start(bias_sb[name][:, :], biasd[name][:, :])
                if name in has_brh:
                    brh_sb[name] = const.tile([128, 128], bf16, tag=f"brh_{name}", name=f"brhsb_{name}")
                    nc.sync.dma_start(brh_sb[name][:, :], brhd[name][:, :])
                else:
                    brh_sb[name] = None

            idx_sb = const.tile([128, ntok // 16], i16)
            nc.sync.dma_start(idx_sb[:, :], xidx[:, :])

            def load_w(dram, nk):
                pool = wpool8 if nk == 8 else wpool
                t_ = pool.tile([128, nk * 12 * 128], bf16,
                               tag=f"W{nk}", name="Wt")
                nc.sync.dma_start(t_[:, :], dram[:, :])
                return t_

            tch = min(16, nT)

            def y_moving(Ytile, koff=0):
                r = Ytile[:, :].rearrange("p (t q b) -> p q t b", q=4, b=B)
                return lambda k, c: r[:, k - koff, c * tch: (c + 1) * tch, :]

            tch = min(16, nT)
            with tc.tile_pool(name="p01", bufs=1) as p01:
                eT = p01.tile([128, 4 * ntok], bf16)
                gch = min(512, ntok)   # tokens per gather call = bulk chunk
                eTr = eT[:, :].rearrange("p (c k n) -> p c k n", k=4, n=gch)
                for gc in range(ntok // gch):
                    nc.gpsimd.dma_gather(
                        eTr[:, gc],
                        E16[:, :],
                        idx_sb[:, gc * (gch // 16): (gc + 1) * (gch // 16)],
                        num_idxs=gch,
                        num_idxs_reg=gch,
                        elem_size=EMB,
                        transpose=True,
                    )
                e_moving = (4, lambda k, c: eTr[:, c, k, :])

                Wfw_t = load_w(Wp["fw"], 4)
                Wbw_t = load_w(Wp["bw"], 4)
                nf, ef = _bulk_xp(nc, tc, pools, Wfw_t, e_moving, Xd["fw"],
                                  bias_sb["fw"][:, :], mask_sb[:, :], nT, "xf")
                nb, eb = _bulk_xp(nc, tc, pools, Wbw_t, e_moving, Xd["bw"],
                                  bias_sb["bw"][:, :], mask_sb[:, :], nT, "xb")
                for c in range(nf):
                    ef(c)
                    eb(c)

            with tc.tile_pool(name="p23", bufs=1) as p23:
                Yfw = p23.tile([128, nT * 128], bf16, tag="Yfw", name="Yfw")
                Ybw = p23.tile([128, nT * 128], bf16, tag="Ybw", name="Ybw")
                Ufw_t = load_w(Up["fw"], 4)
                Ubw_t = load_w(Up["bw"], 4)
                sfw = _scan(nc, tc, pools, Ufw_t, Xd["fw"], Yfw, brh_sb["fw"],
                            ident16, nT, False, "fw")
                sbw = _scan(nc, tc, pools, Ubw_t, Xd["bw"], Ybw, brh_sb["bw"],
                            ident16, nT, True, "bw")

                Wm0_t = load_w(Wp["m0"], 8)
                mf, mb = y_moving(Yfw), y_moving(Ybw, 4)
                nch, em0 = _bulk_xp(
                    nc, tc, pools, Wm0_t,
                    (8, lambda k, c: mf(k, c) if k < 4 else mb(k, c)),
                    Xd["m0"], bias_sb["m0"][:, :], mask_sb[:, :], nT, "x0")
                # xp_m0 chunk c needs fw through step tch*c+tch-1 and bw
                # through its step nT-1-tch*c
                avail = {}
                for c in range(nch):
                    avail.setdefault(
                        max(tch * c + tch - 1, nT - 1 - tch * c), []).append(c)
                for i in range(nT):
                    sfw(i)
                    sbw(i)
                    for c in avail.get(i, ()):
                        em0(c)

            with tc.tile_pool(name="p45", bufs=1) as p45:
                Ym0 = p45.tile([128, nT * 128], bf16, tag="Ym0", name="Ym0")
                Um0_t = load_w(Up["m0"], 4)
                sm0 = _scan(nc, tc, pools, Um0_t, Xd["m0"], Ym0, brh_sb["m0"],
                            ident16, nT, False, "m0")
                Wm1_t = load_w(Wp["m1"], 4)
                nch, em1 = _bulk_xp(nc, tc, pools, Wm1_t, (4, y_moving(Ym0)),
                                    Xd["m1"], bias_sb["m1"][:, :],
                                    mask_sb[:, :], nT, "x1")
                for i in range(nT):
                    sm0(i)
                    if i % tch == tch - 1:
                        em1(i // tch)

            with tc.tile_pool(name="p67", bufs=1) as p67:
                Ym1 = p67.tile([128, nT * 128], bf16, tag="Ym1", name="Ym1")
                Um1_t = load_w(Up["m1"], 4)
                sm1 = _scan(nc, tc, pools, Um1_t, Xd["m1"], Ym1, brh_sb["m1"],
                            ident16, nT, False, "m1")
                Wtop_t = load_w(Wp["top"], 4)
                nch, etp = _bulk_xp(nc, tc, pools, Wtop_t, (4, y_moving(Ym1)),
                                    Xd["top"], bias_sb["top"][:, :],
                                    mask_sb[:, :], nT, "xt")
                for i in range(nT):
                    sm1(i)
                    if i % tch == tch - 1:
                        etp(i // tch)

            with tc.tile_pool(name="p89", bufs=1) as p89:
                Ytop = p89.tile([128, nT * 128], bf16, tag="Ytop", name="Ytop")
                Utop_t = load_w(Up["top"], 4)
                stp = _scan(nc, tc, pools, Utop_t, Xd["top"], Ytop,
                            brh_sb["top"], ident16, nT, False, "top",
                            y_out=y_out, hT_out=hT_out, ident32=ident32)
                for i in range(nT):
                    stp(i)

    nc.finalize()
    return nc


RUN_CORES = 1


def _run_hw(nc, ins):
    from concourse.bass_utils import run_bass_kernel_spmd

    in_maps = [ins for _ in range(RUN_CORES)]
    res = run_bass_kernel_spmd(nc, in_maps, list(range(RUN_CORES)))
    return res.results[0]


def kernel(**inputs):
    nT = T
    has_brh = tuple(sorted(
        name for name in ("fw", "bw", "m0", "m1", "top")
        if np.any(np.asarray(inputs[f"b_{name}"])[1, 1024:] != 0)
    ))
    key = ("nc", nT, has_brh)
    if key not in _CACHE:
        _CACHE[key] = build(nT, has_brh)
    nc = _CACHE[key]
    ins = prep_inputs(inputs, nT)
    out = _run_hw(nc, ins)
    return np.asarray(out["y"], np.float32), np.asarray(out["hT"], np.float32)


# revision 18
# speedup vs baseline: 1.2935x; 1.2850x over previous
"""Trainium2 Bass kernel for a 5-layer GRU encoder (bi-GRU + 3 stacked GRUs).

Layout convention ("transposed"/unit-major):
  - state tiles:  [128 part = unit-sub s, free = 32*q + b]  (q = unit chunk 0..3, b = batch 0..31)
  - PSUM rec:     [128 part = s, free = 128*g + 32*q + b]   (g = gate z/r/h)
  - xp (X) tiles: same free layout as PSUM rec, per timestep, streamed from DRAM
  - token order:  tok = 32*t + b  (time-major)

Per GRU step (Keras reset_after=True, sigmoid activations everywhere):
  z = sig(xz + rz);  r = sig(xr + rr);  hh = sig(xh + r*(rec_h + brec_h))
  hn = z*h + (1-z)*hh ; masked steps carry h.
We pre-negate the z columns of W/U/biases on the host so PSUM holds -(xz+rz)
and one sigmoid yields z' = 1-z; the pad mask is folded into X's z region as
-1e9 so z'=0 (carry) at masked steps:  hn = h + z'*(hh - h).
"""

import numpy as np

VOCAB = 32000
EMB = 512
UNITS = 512
B = 32
T = 128
NCORES = 8

F32 = None  # set lazily (mybir dtypes)
BF16 = None

_CACHE = {}


def _dt():
    import concourse.mybir as mybir

    return mybir.dt.float32, mybir.dt.bfloat16, mybir.dt.int16


# ---------------------------------------------------------------------------
# host-side weight packing
# ---------------------------------------------------------------------------

def _pack_stationary(W, negate_z):
    """W: [din, 3U] -> stationary pack [128, nk*12*128] bf16.

    col ((kd*12 + m)*128 + s), part kp  =  W[128*kd + kp, 512*g + 128*q + s]
    with m = 4*g + q.  z columns (g=0) negated when negate_z.
    """
    din = W.shape[0]
    nk = din // 128
    W = W.astype(np.float32).copy()
    if negate_z:
        W[:, 0:UNITS] = -W[:, 0:UNITS]
    # [din, 1536] -> [nk, 128(kp), g, q, s=128]
    Wr = W.reshape(nk, 128, 3, 4, 128)
    # -> [kp, nk, g*4+q, s]
    Wp = Wr.transpose(1, 0, 2, 3, 4).reshape(128, nk * 12 * 128)
    return _to_bf16(Wp)


def _to_bf16(a):
    import ml_dtypes

    return np.asarray(a, dtype=np.float32).astype(ml_dtypes.bfloat16)


def _gate_bias(b, negate_z):
    """b: [2, 3U] -> (bias_zr_pack [128,12] f32 (z,r: b_in+b_rec ; h: b_in),
                      brh [128,128] bf16 (b_rec_h broadcast))"""
    b_in, b_rec = b[0].astype(np.float32), b[1].astype(np.float32)
    tot = b_in + b_rec            # for z, r
    bias = np.zeros((128, 12), np.float32)
    for g in range(3):
        src = tot if g < 2 else b_in
        for q in range(4):
            col = src[512 * g + 128 * q: 512 * g + 128 * (q + 1)].copy()
            if g == 0 and negate_z:
                col = -col
            bias[:, 4 * g + q] = col
    brh = np.zeros((128, 128), np.float32)
    for q in range(4):
        brh[:, 32 * q: 32 * (q + 1)] = b_rec[1024 + 128 * q: 1024 + 128 * (q + 1)][:, None]
    return bias, _to_bf16(brh)


def prep_inputs(inputs, nT=T):
    """Host-side prep: cast/pack weights, build indices and mask."""
    x = np.asarray(inputs["x"]).astype(np.int64)          # [B, T]
    E = np.asarray(inputs["E"], np.float32)

    ins = {}
    ins["E16"] = _to_bf16(E)

    # token order tok = 32*t + b ; wrapped idx layout [16, ntok/16]
    xt = x.T.reshape(-1)[: B * nT]                        # tok = t*B + b
    idx = np.zeros((128, (B * nT) // 16), np.int16)
    for i in range(B * nT):
        idx[i % 16, i // 16] = np.int16(xt[i])
    ins["xidx"] = idx

    maskneg = np.where(xt == 0, np.float32(-1e9), np.float32(0.0))
    ins["maskneg"] = _to_bf16(np.broadcast_to(maskneg[None, :], (128, B * nT)).copy())

    for name in ("fw", "bw", "m0", "m1", "top"):
        ins[f"W_{name}p"] = _pack_stationary(np.asarray(inputs[f"W_{name}"]), True)
        ins[f"U_{name}p"] = _pack_stationary(np.asarray(inputs[f"U_{name}"]), True)
        bias, brh = _gate_bias(np.asarray(inputs[f"b_{name}"]), True)
        ins[f"bias_{name}"] = bias
        ins[f"brh_{name}"] = brh

    ins["ident16"] = _to_bf16(np.eye(128, dtype=np.float32))
    ins["ident32"] = np.eye(128, dtype=np.float32)
    return ins


# ---------------------------------------------------------------------------
# device program
# ---------------------------------------------------------------------------

def _bulk_xp(nc, tc, pools, Wt, movings, Xd, bias_sb, maskneg_sb, nT, sig):
    """xp for one layer: X[t,s,c] = sum_k in^T @ W  (+bias, +mask on z).

    Wt: stationary pack tile [128, nk*12*128] bf16
    movings: list of nk APs, each [128, ntok] bf16 (k-chunk of input^T)
    Xd: DRAM [nT, 128, 384] bf16
    bias_sb: [128, 12] f32 AP  (per (g,q) block, z pre-negated)
    """
    import concourse.mybir as mybir

    f32, bf16, _ = _dt()
    nk, moving_fn = movings
    tch = min(16, nT)              # timesteps per 512-token chunk
    nch = nT // tch
    csz = tch * B

    def emit_chunk(c):
        for m in range(12):
            P = pools["bulkp"].tile([128, csz], f32, tag="bulkP", name="bulkP")
            for k in range(nk):
                nc.tensor.matmul(
                    P[:, :],
                    Wt[:, (k * 12 + m) * 128: (k * 12 + m + 1) * 128],
                    moving_fn(k, c),
                    start=(k == 0),
                    stop=(k == nk - 1),
                )
            S = pools["bulks"].tile([128, csz], bf16, tag="bulkS", name="bulkS")
            if m < 4:  # z block: bias + mask fold, one DVE op
                nc.vector.scalar_tensor_tensor(
                    S[:, :], P[:, :], bias_sb[:, m: m + 1],
                    maskneg_sb[:, c * csz: (c + 1) * csz],
                    mybir.AluOpType.add, mybir.AluOpType.add,
                )
            else:
                nc.scalar.activation(
                    S[:, :], P[:, :], mybir.ActivationFunctionType.Identity,
                    bias=bias_sb[:, m: m + 1],
                )
            # X[t, s, 32*m : 32*m+32] for the tch timesteps in this chunk
            dst = Xd[c * tch: (c + 1) * tch, :, 32 * m: 32 * (m + 1)]
            nc.sync.dma_start(dst.rearrange("t p b -> p t b"),
                              S[:, :].rearrange("p (t b) -> p t b", b=B))

    return nch, emit_chunk


def _scan(nc, tc, pools, Ut, Xd, Y, brh_sb, ident16, nT, reverse, sig,
          y_out=None, hT_out=None, ident32=None):
    """One GRU scan. Y: SBUF tile [128, nT*128] bf16 output (also next-step state).
    If y_out is given (top layer), also emit transposed per-step outputs."""
    import concourse.mybir as mybir

    f32, bf16, _ = _dt()
    Sig = mybir.ActivationFunctionType.Sigmoid

    h_prev = pools["state"].tile([128, 128], f32, tag=f"h{sig}", name=f"h{sig}")
    nc.vector.memset(h_prev[:, :], 0.0)

    order = list(range(nT - 1, -1, -1) if reverse else range(nT))
    state = {"h_prev": h_prev, "tprev": None, "first": True}

    def emit_step(i):
        t = order[i]
        h_prev = state["h_prev"]
        tprev = state["tprev"]
        first = state["first"]
        Xt = pools["xstream"].tile([128, 384], bf16, tag="Xt", name="Xt")
        nc.sync.dma_start(Xt[:, :], Xd[t])
        R = pools["scanp"].tile([128, 384], f32, tag="R", name="R")
        # inject all of xp into PSUM via identity matmul (single start per bank)
        nc.tensor.matmul(R[:, 0:384], ident16[:, :], Xt[:, 0:384],
                         start=True, stop=first)
        if not first:
            for kq in range(4):
                rhs = Y[:, 128 * tprev + 32 * kq: 128 * tprev + 32 * (kq + 1)]
                for m in range(12):
                    nc.tensor.matmul(
                        R[:, 32 * m: 32 * (m + 1)],
                        Ut[:, (kq * 12 + m) * 128: (kq * 12 + m + 1) * 128],
                        rhs,
                        start=False,
                        stop=(kq == 3 and m == 11),
                    )
        # gates: PSUM holds [-(xz+rz) | xr+rr | xh+rec_h]
        zr = pools["gates"].tile([128, 256], f32, tag="zr", name="zr")
        nc.scalar.activation(zr[:, :], R[:, 0:256], Sig)
        a = pools["gates"].tile([128, 128], f32, tag="a", name="a")
        nc.vector.tensor_sub(a[:, :], R[:, 256:384], Xt[:, 256:384])
        if brh_sb is not None:
            a2 = pools["gates"].tile([128, 128], f32, tag="a2", name="a2")
            nc.vector.tensor_add(a2[:, :], a[:, :], brh_sb[:, :])
            a = a2
        t1 = pools["gates"].tile([128, 128], f32, tag="t1", name="t1")
        nc.vector.tensor_mul(t1[:, :], zr[:, 128:256], a[:, :])
        hp = pools["gates"].tile([128, 128], f32, tag="hp", name="hp")
        nc.vector.tensor_add(hp[:, :], t1[:, :], Xt[:, 256:384])
        hh = pools["gates"].tile([128, 128], f32, tag="hh", name="hh")
        nc.scalar.activation(hh[:, :], hp[:, :], Sig)
        d = pools["gates"].tile([128, 128], f32, tag="d", name="d")
        nc.vector.tensor_sub(d[:, :], hh[:, :], h_prev[:, :])
        wd = pools["gates"].tile([128, 128], f32, tag="wd", name="wd")
        nc.vector.tensor_mul(wd[:, :], zr[:, 0:128], d[:, :])
        h_new = pools["state"].tile([128, 128], f32, tag=f"h{sig}", name=f"h{sig}")
        nc.vector.tensor_add(h_new[:, :], h_prev[:, :], wd[:, :])
        nc.vector.tensor_copy(Y[:, 128 * t: 128 * (t + 1)], h_new[:, :])

        if y_out is not None:
            Py = pools["ypsum"].tile([32, 512], f32, tag="Py", name="Py")
            for q in range(4):
                nc.tensor.transpose(Py[:, 128 * q: 128 * (q + 1)],
                                    h_new[:, 32 * q: 32 * (q + 1)],
                                    ident32[:, :])
            ys = pools["ystage"].tile([32, 512], f32, tag="ys", name="ys")
            nc.scalar.activation(ys[:, :], Py[:, :],
                                 mybir.ActivationFunctionType.Copy)
            nc.sync.dma_start(y_out[:, t, :], ys[:, :])
            if hT_out is not None and t == nT - 1:
                nc.sync.dma_start(hT_out[:, :], ys[:, :])

        state["h_prev"] = h_new
        state["tprev"] = t
        state["first"] = False

    return emit_step


def build(nT=T, has_brh=()):
    import concourse.bacc as bacc
    import concourse.mybir as mybir
    import concourse.tile as tile

    f32, bf16, i16 = _dt()
    ntok = B * nT

    nc = bacc.Bacc(None, target_bir_lowering=False,
                   dynamic_dma_scratch_size=2 ** 16)

    # I/O
    E16 = nc.dram_tensor("E16", [VOCAB, EMB], bf16, kind="ExternalInput")
    xidx = nc.dram_tensor("xidx", [128, ntok // 16], i16, kind="ExternalInput")
    maskneg = nc.dram_tensor("maskneg", [128, ntok], bf16, kind="ExternalInput")
    Wp = {}
    for name, nk in (("fw", 4), ("bw", 4), ("m0", 8), ("m1", 4), ("top", 4)):
        Wp[name] = nc.dram_tensor(f"W_{name}p", [128, nk * 12 * 128], bf16,
                                  kind="ExternalInput")
    Up = {name: nc.dram_tensor(f"U_{name}p", [128, 4 * 12 * 128], bf16,
                               kind="ExternalInput")
          for name in ("fw", "bw", "m0", "m1", "top")}
    biasd = {name: nc.dram_tensor(f"bias_{name}", [128, 12], f32,
                                  kind="ExternalInput")
             for name in ("fw", "bw", "m0", "m1", "top")}
    brhd = {name: nc.dram_tensor(f"brh_{name}", [128, 128], bf16,
                                 kind="ExternalInput")
            for name in ("fw", "bw", "m0", "m1", "top")}
    ident16_d = nc.dram_tensor("ident16", [128, 128], bf16, kind="ExternalInput")
    ident32_d = nc.dram_tensor("ident32", [128, 128], f32, kind="ExternalInput")

    y_out = nc.dram_tensor("y", [B, nT, UNITS], f32, kind="ExternalOutput")
    hT_out = nc.dram_tensor("hT", [B, UNITS], f32, kind="ExternalOutput")

    # internal DRAM xp buffers
    Xd = {name: nc.dram_tensor(f"X_{name}", [nT, 128, 384], bf16)
          for name in ("fw", "bw", "m0", "m1", "top")}

    with tile.TileContext(nc) as tc:
        import contextlib

        with contextlib.ExitStack() as stk:
            const = stk.enter_context(tc.tile_pool(name="const", bufs=1))
            wpool = stk.enter_context(tc.tile_pool(name="wpool", bufs=2))
            wpool8 = stk.enter_context(tc.tile_pool(name="wpool8", bufs=1))
            pools = {
                "bulkp": stk.enter_context(
                    tc.tile_pool(name="bulkp", bufs=3, space="PSUM")),
                "bulks": stk.enter_context(tc.tile_pool(name="bulks", bufs=3)),
                "scanp": stk.enter_context(
                    tc.tile_pool(name="scanp", bufs=2, space="PSUM")),
                "xstream": stk.enter_context(tc.tile_pool(name="xstream", bufs=3)),
                "gates": stk.enter_context(tc.tile_pool(name="gates", bufs=2)),
                "state": stk.enter_context(tc.tile_pool(name="state", bufs=2)),
                "ypsum": stk.enter_context(
                    tc.tile_pool(name="ypsum", bufs=2, space="PSUM")),
                "ystage": stk.enter_context(tc.tile_pool(name="ystage", bufs=2)),
            }

            ident16 = const.tile([128, 128], bf16)
            nc.sync.dma_start(ident16[:, :], ident16_d[:, :])
            ident32 = const.tile([128, 128], f32)
            nc.sync.dma_start(ident32[:, :], ident32_d[:, :])
            mask_sb = const.tile([128, ntok], bf16)
            nc.sync.dma_start(mask_sb[:, :], maskneg[:, :])
            bias_sb = {}
            brh_sb = {}
            for name in ("fw", "bw", "m0", "m1", "top"):
                bias_sb[name] = const.tile([128, 12], f32, tag=f"bias_{name}", name=f"biassb_{name}")
                nc.sync.dma_start(bias_sb[name][:, :], biasd[name][:, :])
                if name in has_brh:
                    brh_sb[name] = const.tile([128, 128], bf16, tag=f"brh_{name}", name=f"brhsb_{name}")
                    nc.sync.dma_start(brh_sb[name][:, :], brhd[name][:, :])
                else:
                    brh_sb[name] = None

            idx_sb = const.tile([128, ntok // 16], i16)
            nc.sync.dma_start(idx_sb[:, :], xidx[:, :])

            def load_w(dram, nk):
                pool = wpool8 if nk == 8 else wpool
                t_ = pool.tile([128, nk * 12 * 128], bf16,
                               tag=f"W{nk}", name="Wt")
                nc.sync.dma_start(t_[:, :], dram[:, :])
                return t_

            tch = min(16, nT)

            def y_moving(Ytile, koff=0):
                r = Ytile[:, :].rearrange("p (t q b) -> p q t b", q=4, b=B)
                return lambda k, c: r[:, k - koff, c * tch: (c + 1) * tch, :]

            tch = min(16, nT)
            with tc.tile_pool(name="p01", bufs=1) as p01:
                eT = p01.tile([128, 4 * ntok], bf16)
                gch = min(512, ntok)   # tokens per gather call = bulk chunk
                eTr = eT[:, :].rearrange("p (c k n) -> p c k n", k=4, n=gch)
                for gc in range(ntok // gch):
                    nc.gpsimd.dma_gather(
                        eTr[:, gc],
                        E16[:, :],
                        idx_sb[:, gc * (gch // 16): (gc + 1) * (gch // 16)],
                        num_idxs=gch,
                        num_idxs_reg=gch,
                        elem_size=EMB,
                        transpose=True,
                    )
                e_moving = (4, lambda k, c: eTr[:, c, k, :])

                Wfw_t = load_w(Wp["fw"], 4)
                Wbw_t = load_w(Wp["bw"], 4)
                nf, ef = _bulk_xp(nc, tc, pools, Wfw_t, e_moving, Xd["fw"],
                                  bias_sb["fw"][:, :], mask_sb[:, :], nT, "xf")
                nb, eb = _bulk_xp(nc, tc, pools, Wbw_t, e_moving, Xd["bw"],
                                  bias_sb["bw"][:, :], mask_sb[:, :], nT, "xb")
                for c in range(nf):
                    ef(c)
                    eb(c)

            with tc.tile_pool(name="p23", bufs=1) as p23:
                Yfw = p23.tile([128, nT * 128], bf16, tag="Yfw", name="Yfw")
                Ybw = p23.tile([128, nT * 128], bf16, tag="Ybw", name="Ybw")
                Ufw_t = load_w(Up["fw"], 4)
                Ubw_t = load_w(Up["bw"], 4)
                sfw = _scan(nc, tc, pools, Ufw_t, Xd["fw"], Yfw, brh_sb["fw"],
                            ident16, nT, False, "fw")
                sbw = _scan(nc, tc, pools, Ubw_t, Xd["bw"], Ybw, brh_sb["bw"],
                            ident16, nT, True, "bw")

                Wm0_t = load_w(Wp["m0"], 8)
                mf, mb = y_moving(Yfw), y_moving(Ybw, 4)
                nch, em0 = _bulk_xp(
                    nc, tc, pools, Wm0_t,
                    (8, lambda k, c: mf(k, c) if k < 4 else mb(k, c)),
                    Xd["m0"], bias_sb["m0"][:, :], mask_sb[:, :], nT, "x0")
                # xp_m0 chunk c needs fw through step tch*c+tch-1 and bw
                # through its step nT-1-tch*c
                avail = {}
                for c in range(nch):
                    avail.setdefault(
                        max(tch * c + tch - 1, nT - 1 - tch * c), []).append(c)
                for i in range(nT):
                    sfw(i)
                    sbw(i)
                    for c in avail.get(i, ()):
                        em0(c)

            with tc.tile_pool(name="p45", bufs=1) as p45:
                Ym0 = p45.tile([128, nT * 128], bf16, tag="Ym0", name="Ym0")
                Um0_t = load_w(Up["m0"], 4)
                sm0 = _scan(nc, tc, pools, Um0_t, Xd["m0"], Ym0, brh_sb["m0"],
                            ident16, nT, False, "m0")
                Wm1_t = load_w(Wp["m1"], 4)
                nch, em1 = _bulk_xp(nc, tc, pools, Wm1_t, (4, y_moving(Ym0)),
                                    Xd["m1"], bias_sb["m1"][:, :],
                                    mask_sb[:, :], nT, "x1")
                for i in range(nT):
                    sm0(i)
                    if i % tch == tch - 1:
                        em1(i // tch)

            with tc.tile_pool(name="p67", bufs=1) as p67:
                Ym1 = p67.tile([128, nT * 128], bf16, tag="Ym1", name="Ym1")
                Um1_t = load_w(Up["m1"], 4)
                sm1 = _scan(nc, tc, pools, Um1_t, Xd["m1"], Ym1, brh_sb["m1"],
                            ident16, nT, False, "m1")
                Wtop_t = load_w(Wp["top"], 4)
                nch, etp = _bulk_xp(nc, tc, pools, Wtop_t, (4, y_moving(Ym1)),
                                    Xd["top"], bias_sb["top"][:, :],
                                    mask_sb[:, :], nT, "xt")
                for i in range(nT):
                    sm1(i)
                    if i % tch == tch - 1:
                        etp(i // tch)

            with tc.tile_pool(name="p89", bufs=1) as p89:
                Ytop = p89.tile([128, nT * 128], bf16, tag="Ytop", name="Ytop")
                Utop_t = load_w(Up["top"], 4)
                stp = _scan(nc, tc, pools, Utop_t, Xd["top"], Ytop,
                            brh_sb["top"], ident16, nT, False, "top",
                            y_out=y_out, hT_out=hT_out, ident32=ident32)
                for i in range(nT):
                    stp(i)

    nc.finalize()
    return nc


RUN_CORES = 1


def _run_hw(nc, ins):
    from concourse.bass_utils import run_bass_kernel_spmd

    in_maps = [ins for _ in range(RUN_CORES)]
    res = run_bass_kernel_spmd(nc, in_maps, list(range(RUN_CORES)))
    return res.results[0]


def _brh_key(inputs):
    return tuple(sorted(
        name for name in ("fw", "bw", "m0", "m1", "top")
        if np.any(np.asarray(inputs[f"b_{name}"])[1, 1024:] != 0)
    ))


def kernel(**inputs):
    nT = T
    has_brh = _brh_key(inputs)
    key = ("nc", nT, has_brh)
    if key not in _CACHE:
        _CACHE[key] = build(nT, has_brh)
    nc = _CACHE[key]
    ins = prep_inputs(inputs, nT)
    out = _run_hw(nc, ins)
    return np.asarray(out["y"], np.float32), np.asarray(out["hT"], np.float32)


# revision 20
# speedup vs baseline: 1.3022x; 1.0067x over previous
"""Trainium2 Bass kernel for a 5-layer GRU encoder (bi-GRU + 3 stacked GRUs).

Layout convention ("transposed"/unit-major):
  - state tiles:  [128 part = unit-sub s, free = 32*q + b]  (q = unit chunk 0..3, b = batch 0..31)
  - PSUM rec:     [128 part = s, free = 128*g + 32*q + b]   (g = gate z/r/h)
  - xp (X) tiles: same free layout as PSUM rec, per timestep, streamed from DRAM
  - token order:  tok = 32*t + b  (time-major)

Per GRU step (Keras reset_after=True, sigmoid activations everywhere):
  z = sig(xz + rz);  r = sig(xr + rr);  hh = sig(xh + r*(rec_h + brec_h))
  hn = z*h + (1-z)*hh ; masked steps carry h.
We pre-negate the z columns of W/U/biases on the host so PSUM holds -(xz+rz)
and one sigmoid yields z' = 1-z; the pad mask is folded into X's z region as
-1e9 so z'=0 (carry) at masked steps:  hn = h + z'*(hh - h).

Sharding choice: the 128-step recurrences of the 5 GRU layers dominate the
critical path and are strictly sequential (the bidirectional merge forces the
backward scan to fully complete before layer 2 can begin), so batch-32
data-parallelism buys nothing on the scan (matmul cycles are K*N-bound,
batch-independent).  We therefore run the whole fused network on core 0 via
run_bass_kernel_spmd (cores 1-7 idle); input-projection GEMMs are computed in
bulk (tokens-on-moving-dim, 4x better PE utilisation than per-step) and
interleaved into the scans' gate-latency bubbles.
"""

import numpy as np

VOCAB = 32000
EMB = 512
UNITS = 512
B = 32
T = 128
NCORES = 8

F32 = None  # set lazily (mybir dtypes)
BF16 = None

_CACHE = {}


def _dt():
    import concourse.mybir as mybir

    return mybir.dt.float32, mybir.dt.bfloat16, mybir.dt.int16


# ---------------------------------------------------------------------------
# host-side weight packing
# ---------------------------------------------------------------------------

def _pack_stationary(W, negate_z):
    """W: [din, 3U] -> stationary pack [128, nk*12*128] bf16.

    col ((kd*12 + m)*128 + s), part kp  =  W[128*kd + kp, 512*g + 128*q + s]
    with m = 4*g + q.  z columns (g=0) negated when negate_z.
    """
    din = W.shape[0]
    nk = din // 128
    W = W.astype(np.float32).copy()
    if negate_z:
        W[:, 0:UNITS] = -W[:, 0:UNITS]
    # [din, 1536] -> [nk, 128(kp), g, q, s=128]
    Wr = W.reshape(nk, 128, 3, 4, 128)
    # -> [kp, nk, g*4+q, s]
    Wp = Wr.transpose(1, 0, 2, 3, 4).reshape(128, nk * 12 * 128)
    return _to_bf16(Wp)


def _to_bf16(a):
    import ml_dtypes

    return np.asarray(a, dtype=np.float32).astype(ml_dtypes.bfloat16)


def _gate_bias(b, negate_z):
    """b: [2, 3U] -> (bias_zr_pack [128,12] f32 (z,r: b_in+b_rec ; h: b_in),
                      brh [128,128] bf16 (b_rec_h broadcast))"""
    b_in, b_rec = b[0].astype(np.float32), b[1].astype(np.float32)
    tot = b_in + b_rec            # for z, r
    bias = np.zeros((128, 12), np.float32)
    for g in range(3):
        src = tot if g < 2 else b_in
        for q in range(4):
            col = src[512 * g + 128 * q: 512 * g + 128 * (q + 1)].copy()
            if g == 0 and negate_z:
                col = -col
            bias[:, 4 * g + q] = col
    brh = np.zeros((128, 128), np.float32)
    for q in range(4):
        brh[:, 32 * q: 32 * (q + 1)] = b_rec[1024 + 128 * q: 1024 + 128 * (q + 1)][:, None]
    return bias, _to_bf16(brh)


def prep_inputs(inputs, nT=T):
    """Host-side prep: cast/pack weights, build indices and mask."""
    x = np.asarray(inputs["x"]).astype(np.int64)          # [B, T]
    E = np.asarray(inputs["E"], np.float32)

    ins = {}
    ins["E16"] = _to_bf16(E)

    # token order tok = 32*t + b ; wrapped idx layout [16, ntok/16]
    xt = x.T.reshape(-1)[: B * nT]                        # tok = t*B + b
    idx = np.zeros((128, (B * nT) // 16), np.int16)
    for i in range(B * nT):
        idx[i % 16, i // 16] = np.int16(xt[i])
    ins["xidx"] = idx

    maskneg = np.where(xt == 0, np.float32(-1e9), np.float32(0.0))
    ins["maskneg"] = _to_bf16(np.broadcast_to(maskneg[None, :], (128, B * nT)).copy())

    for name in ("fw", "bw", "m0", "m1", "top"):
        ins[f"W_{name}p"] = _pack_stationary(np.asarray(inputs[f"W_{name}"]), True)
        ins[f"U_{name}p"] = _pack_stationary(np.asarray(inputs[f"U_{name}"]), True)
        bias, brh = _gate_bias(np.asarray(inputs[f"b_{name}"]), True)
        ins[f"bias_{name}"] = bias
        ins[f"brh_{name}"] = brh

    ins["ident16"] = _to_bf16(np.eye(128, dtype=np.float32))
    ins["ident32"] = np.eye(128, dtype=np.float32)
    return ins


# ---------------------------------------------------------------------------
# device program
# ---------------------------------------------------------------------------

def _bulk_xp(nc, tc, pools, Wt, movings, Xd, bias_sb, maskneg_sb, nT, sig):
    """xp for one layer: X[t,s,c] = sum_k in^T @ W  (+bias, +mask on z).

    Wt: stationary pack tile [128, nk*12*128] bf16
    movings: list of nk APs, each [128, ntok] bf16 (k-chunk of input^T)
    Xd: DRAM [nT, 128, 384] bf16
    bias_sb: [128, 12] f32 AP  (per (g,q) block, z pre-negated)
    """
    import concourse.mybir as mybir

    f32, bf16, _ = _dt()
    nk, moving_fn = movings
    tch = min(16, nT)              # timesteps per 512-token chunk
    nch = nT // tch
    csz = tch * B

    def emit_chunk(c):
        for m in range(12):
            P = pools["bulkp"].tile([128, csz], f32, tag="bulkP", name="bulkP")
            for k in range(nk):
                nc.tensor.matmul(
                    P[:, :],
                    Wt[:, (k * 12 + m) * 128: (k * 12 + m + 1) * 128],
                    moving_fn(k, c),
                    start=(k == 0),
                    stop=(k == nk - 1),
                )
            S = pools["bulks"].tile([128, csz], bf16, tag="bulkS", name="bulkS")
            if m < 4:  # z block: bias + mask fold, one DVE op
                nc.vector.scalar_tensor_tensor(
                    S[:, :], P[:, :], bias_sb[:, m: m + 1],
                    maskneg_sb[:, c * csz: (c + 1) * csz],
                    mybir.AluOpType.add, mybir.AluOpType.add,
                )
            else:
                nc.scalar.activation(
                    S[:, :], P[:, :], mybir.ActivationFunctionType.Identity,
                    bias=bias_sb[:, m: m + 1],
                )
            # X[t, s, 32*m : 32*m+32] for the tch timesteps in this chunk
            dst = Xd[c * tch: (c + 1) * tch, :, 32 * m: 32 * (m + 1)]
            nc.sync.dma_start(dst.rearrange("t p b -> p t b"),
                              S[:, :].rearrange("p (t b) -> p t b", b=B))

    return nch, emit_chunk


def _scan(nc, tc, pools, Ut, Xd, Y, brh_sb, ident16, nT, reverse, sig,
          y_out=None, hT_out=None, ident32=None):
    """One GRU scan. Y: SBUF tile [128, nT*128] bf16 output (also next-step state).
    If y_out is given (top layer), also emit transposed per-step outputs."""
    import concourse.mybir as mybir

    f32, bf16, _ = _dt()
    Sig = mybir.ActivationFunctionType.Sigmoid

    h_prev = pools["state"].tile([128, 128], f32, tag=f"h{sig}", name=f"h{sig}")
    nc.vector.memset(h_prev[:, :], 0.0)

    order = list(range(nT - 1, -1, -1) if reverse else range(nT))
    state = {"h_prev": h_prev, "tprev": None, "first": True}

    def emit_step(i):
        t = order[i]
        h_prev = state["h_prev"]
        tprev = state["tprev"]
        first = state["first"]
        Xt = pools["xstream"].tile([128, 384], bf16, tag="Xt", name="Xt")
        nc.sync.dma_start(Xt[:, :], Xd[t])
        R = pools["scanp"].tile([128, 384], f32, tag="R", name="R")
        # inject all of xp into PSUM via identity matmul (single start per bank)
        nc.tensor.matmul(R[:, 0:384], ident16[:, :], Xt[:, 0:384],
                         start=True, stop=first)
        if not first:
            for kq in range(4):
                rhs = Y[:, 128 * tprev + 32 * kq: 128 * tprev + 32 * (kq + 1)]
                for m in range(12):
                    nc.tensor.matmul(
                        R[:, 32 * m: 32 * (m + 1)],
                        Ut[:, (kq * 12 + m) * 128: (kq * 12 + m + 1) * 128],
                        rhs,
                        start=False,
                        stop=(kq == 3 and m == 11),
                    )
        # gates: PSUM holds [-(xz+rz) | xr+rr | xh+rec_h]
        zr = pools["gates"].tile([128, 256], f32, tag="zr", name="zr")
        nc.scalar.activation(zr[:, :], R[:, 0:256], Sig)
        a = pools["gates"].tile([128, 128], f32, tag="a", name="a")
        nc.vector.tensor_sub(a[:, :], R[:, 256:384], Xt[:, 256:384])
        if brh_sb is not None:
            a2 = pools["gates"].tile([128, 128], f32, tag="a2", name="a2")
            nc.vector.tensor_add(a2[:, :], a[:, :], brh_sb[:, :])
            a = a2
        t1 = pools["gates"].tile([128, 128], f32, tag="t1", name="t1")
        nc.vector.tensor_mul(t1[:, :], zr[:, 128:256], a[:, :])
        hp = pools["gates"].tile([128, 128], f32, tag="hp", name="hp")
        nc.vector.tensor_add(hp[:, :], t1[:, :], Xt[:, 256:384])
        hh = pools["gates"].tile([128, 128], f32, tag="hh", name="hh")
        nc.scalar.activation(hh[:, :], hp[:, :], Sig)
        d = pools["gates"].tile([128, 128], f32, tag="d", name="d")
        nc.vector.tensor_sub(d[:, :], hh[:, :], h_prev[:, :])
        wd = pools["gates"].tile([128, 128], f32, tag="wd", name="wd")
        nc.vector.tensor_mul(wd[:, :], zr[:, 0:128], d[:, :])
        h_new = pools["state"].tile([128, 128], f32, tag=f"h{sig}", name=f"h{sig}")
        nc.vector.tensor_add(h_new[:, :], h_prev[:, :], wd[:, :])
        nc.vector.tensor_copy(Y[:, 128 * t: 128 * (t + 1)], h_new[:, :])

        if y_out is not None:
            Py = pools["ypsum"].tile([32, 512], f32, tag="Py", name="Py")
            for q in range(4):
                nc.tensor.transpose(Py[:, 128 * q: 128 * (q + 1)],
                                    h_new[:, 32 * q: 32 * (q + 1)],
                                    ident32[:, :])
            ys = pools["ystage"].tile([32, 512], f32, tag="ys", name="ys")
            nc.scalar.activation(ys[:, :], Py[:, :],
                                 mybir.ActivationFunctionType.Copy)
            nc.sync.dma_start(y_out[:, t, :], ys[:, :])
            if hT_out is not None and t == nT - 1:
                nc.sync.dma_start(hT_out[:, :], ys[:, :])

        state["h_prev"] = h_new
        state["tprev"] = t
        state["first"] = False

    return emit_step


def build(nT=T, has_brh=()):
    import concourse.bacc as bacc
    import concourse.mybir as mybir
    import concourse.tile as tile

    f32, bf16, i16 = _dt()
    ntok = B * nT

    nc = bacc.Bacc(None, target_bir_lowering=False,
                   dynamic_dma_scratch_size=2 ** 16)

    # I/O
    E16 = nc.dram_tensor("E16", [VOCAB, EMB], bf16, kind="ExternalInput")
    xidx = nc.dram_tensor("xidx", [128, ntok // 16], i16, kind="ExternalInput")
    maskneg = nc.dram_tensor("maskneg", [128, ntok], bf16, kind="ExternalInput")
    Wp = {}
    for name, nk in (("fw", 4), ("bw", 4), ("m0", 8), ("m1", 4), ("top", 4)):
        Wp[name] = nc.dram_tensor(f"W_{name}p", [128, nk * 12 * 128], bf16,
                                  kind="ExternalInput")
    Up = {name: nc.dram_tensor(f"U_{name}p", [128, 4 * 12 * 128], bf16,
                               kind="ExternalInput")
          for name in ("fw", "bw", "m0", "m1", "top")}
    biasd = {name: nc.dram_tensor(f"bias_{name}", [128, 12], f32,
                                  kind="ExternalInput")
             for name in ("fw", "bw", "m0", "m1", "top")}
    brhd = {name: nc.dram_tensor(f"brh_{name}", [128, 128], bf16,
                                 kind="ExternalInput")
            for name in ("fw", "bw", "m0", "m1", "top")}
    ident16_d = nc.dram_tensor("ident16", [128, 128], bf16, kind="ExternalInput")
    ident32_d = nc.dram_tensor("ident32", [128, 128], f32, kind="ExternalInput")

    y_out = nc.dram_tensor("y", [B, nT, UNITS], f32, kind="ExternalOutput")
    hT_out = nc.dram_tensor("hT", [B, UNITS], f32, kind="ExternalOutput")

    # internal DRAM xp buffers
    Xd = {name: nc.dram_tensor(f"X_{name}", [nT, 128, 384], bf16)
          for name in ("fw", "bw", "m0", "m1", "top")}

    with tile.TileContext(nc) as tc:
        import contextlib

        with contextlib.ExitStack() as stk:
            const = stk.enter_context(tc.tile_pool(name="const", bufs=1))
            wpool = stk.enter_context(tc.tile_pool(name="wpool", bufs=2))
            wpool8 = stk.enter_context(tc.tile_pool(name="wpool8", bufs=1))
            pools = {
                "bulkp": stk.enter_context(
                    tc.tile_pool(name="bulkp", bufs=3, space="PSUM")),
                "bulks": stk.enter_context(tc.tile_pool(name="bulks", bufs=3)),
                "scanp": stk.enter_context(
                    tc.tile_pool(name="scanp", bufs=3, space="PSUM")),
                "xstream": stk.enter_context(tc.tile_pool(name="xstream", bufs=4)),
                "gates": stk.enter_context(tc.tile_pool(name="gates", bufs=3)),
                "state": stk.enter_context(tc.tile_pool(name="state", bufs=2)),
                "ypsum": stk.enter_context(
                    tc.tile_pool(name="ypsum", bufs=2, space="PSUM")),
                "ystage": stk.enter_context(tc.tile_pool(name="ystage", bufs=2)),
            }

            ident16 = const.tile([128, 128], bf16)
            nc.sync.dma_start(ident16[:, :], ident16_d[:, :])
            ident32 = const.tile([128, 128], f32)
            nc.sync.dma_start(ident32[:, :], ident32_d[:, :])
            mask_sb = const.tile([128, ntok], bf16)
            nc.sync.dma_start(mask_sb[:, :], maskneg[:, :])
            bias_sb = {}
            brh_sb = {}
            for name in ("fw", "bw", "m0", "m1", "top"):
                bias_sb[name] = const.tile([128, 12], f32, tag=f"bias_{name}", name=f"biassb_{name}")
                nc.sync.dma_start(bias_sb[name][:, :], biasd[name][:, :])
                if name in has_brh:
                    brh_sb[name] = const.tile([128, 128], bf16, tag=f"brh_{name}", name=f"brhsb_{name}")
                    nc.sync.dma_start(brh_sb[name][:, :], brhd[name][:, :])
                else:
                    brh_sb[name] = None

            idx_sb = const.tile([128, ntok // 16], i16)
            nc.sync.dma_start(idx_sb[:, :], xidx[:, :])

            def load_w(dram, nk):
                pool = wpool8 if nk == 8 else wpool
                t_ = pool.tile([128, nk * 12 * 128], bf16,
                               tag=f"W{nk}", name="Wt")
                nc.sync.dma_start(t_[:, :], dram[:, :])
                return t_

            tch = min(16, nT)

            def y_moving(Ytile, koff=0):
                r = Ytile[:, :].rearrange("p (t q b) -> p q t b", q=4, b=B)
                return lambda k, c: r[:, k - koff, c * tch: (c + 1) * tch, :]

            tch = min(16, nT)
            with tc.tile_pool(name="p01", bufs=1) as p01:
                eT = p01.tile([128, 4 * ntok], bf16)
                gch = min(512, ntok)   # tokens per gather call = bulk chunk
                eTr = eT[:, :].rearrange("p (c k n) -> p c k n", k=4, n=gch)
                for gc in range(ntok // gch):
                    nc.gpsimd.dma_gather(
                        eTr[:, gc],
                        E16[:, :],
                        idx_sb[:, gc * (gch // 16): (gc + 1) * (gch // 16)],
                        num_idxs=gch,
                        num_idxs_reg=gch,
                        elem_size=EMB,
                        transpose=True,
                    )
                e_moving = (4, lambda k, c: eTr[:, c, k, :])

                Wfw_t = load_w(Wp["fw"], 4)
                Wbw_t = load_w(Wp["bw"], 4)
                nf, ef = _bulk_xp(nc, tc, pools, Wfw_t, e_moving, Xd["fw"],
                                  bias_sb["fw"][:, :], mask_sb[:, :], nT, "xf")
                nb, eb = _bulk_xp(nc, tc, pools, Wbw_t, e_moving, Xd["bw"],
                                  bias_sb["bw"][:, :], mask_sb[:, :], nT, "xb")
                for c in range(nf):
                    ef(c)
                    eb(c)

            with tc.tile_pool(name="p23", bufs=1) as p23:
                Yfw = p23.tile([128, nT * 128], bf16, tag="Yfw", name="Yfw")
                Ybw = p23.tile([128, nT * 128], bf16, tag="Ybw", name="Ybw")
                Ufw_t = load_w(Up["fw"], 4)
                Ubw_t = load_w(Up["bw"], 4)
                sfw = _scan(nc, tc, pools, Ufw_t, Xd["fw"], Yfw, brh_sb["fw"],
                            ident16, nT, False, "fw")
                sbw = _scan(nc, tc, pools, Ubw_t, Xd["bw"], Ybw, brh_sb["bw"],
                            ident16, nT, True, "bw")

                Wm0_t = load_w(Wp["m0"], 8)
                mf, mb = y_moving(Yfw), y_moving(Ybw, 4)
                nch, em0 = _bulk_xp(
                    nc, tc, pools, Wm0_t,
                    (8, lambda k, c: mf(k, c) if k < 4 else mb(k, c)),
                    Xd["m0"], bias_sb["m0"][:, :], mask_sb[:, :], nT, "x0")
                # xp_m0 chunk c needs fw through step tch*c+tch-1 and bw
                # through its step nT-1-tch*c
                avail = {}
                for c in range(nch):
                    avail.setdefault(
                        max(tch * c + tch - 1, nT - 1 - tch * c), []).append(c)
                for i in range(nT):
                    sfw(i)
                    sbw(i)
                    for c in avail.get(i, ()):
                        em0(c)

            with tc.tile_pool(name="p45", bufs=1) as p45:
                Ym0 = p45.tile([128, nT * 128], bf16, tag="Ym0", name="Ym0")
                Um0_t = load_w(Up["m0"], 4)
                sm0 = _scan(nc, tc, pools, Um0_t, Xd["m0"], Ym0, brh_sb["m0"],
                            ident16, nT, False, "m0")
                Wm1_t = load_w(Wp["m1"], 4)
                nch, em1 = _bulk_xp(nc, tc, pools, Wm1_t, (4, y_moving(Ym0)),
                                    Xd["m1"], bias_sb["m1"][:, :],
                                    mask_sb[:, :], nT, "x1")
                for i in range(nT):
                    sm0(i)
                    if i % tch == tch - 1:
                        em1(i // tch)

            with tc.tile_pool(name="p67", bufs=1) as p67:
                Ym1 = p67.tile([128, nT * 128], bf16, tag="Ym1", name="Ym1")
                Um1_t = load_w(Up["m1"], 4)
                sm1 = _scan(nc, tc, pools, Um1_t, Xd["m1"], Ym1, brh_sb["m1"],
                            ident16, nT, False, "m1")
                Wtop_t = load_w(Wp["top"], 4)
                nch, etp = _bulk_xp(nc, tc, pools, Wtop_t, (4, y_moving(Ym1)),
                                    Xd["top"], bias_sb["top"][:, :],
                                    mask_sb[:, :], nT, "xt")
                for i in range(nT):
                    sm1(i)
                    if i % tch == tch - 1:
                        etp(i // tch)

            with tc.tile_pool(name="p89", bufs=1) as p89:
                Ytop = p89.tile([128, nT * 128], bf16, tag="Ytop", name="Ytop")
                Utop_t = load_w(Up["top"], 4)
                stp = _scan(nc, tc, pools, Utop_t, Xd["top"], Ytop,
                            brh_sb["top"], ident16, nT, False, "top",
                            y_out=y_out, hT_out=hT_out, ident32=ident32)
                for i in range(nT):
                    stp(i)

    nc.finalize()
    return nc


RUN_CORES = 1


def _run_hw(nc, ins):
    from concourse.bass_utils import run_bass_kernel_spmd

    in_maps = [ins for _ in range(RUN_CORES)]
    res = run_bass_kernel_spmd(nc, in_maps, list(range(RUN_CORES)))
    return res.results[0]


def _brh_key(inputs):
    return tuple(sorted(
        name for name in ("fw", "bw", "m0", "m1", "top")
        if np.any(np.asarray(inputs[f"b_{name}"])[1, 1024:] != 0)
    ))


def kernel(**inputs):
    nT = T
    has_brh = _brh_key(inputs)
    key = ("nc", nT, has_brh)
    if key not in _CACHE:
        _CACHE[key] = build(nT, has_brh)
    nc = _CACHE[key]
    ins = prep_inputs(inputs, nT)
    out = _run_hw(nc, ins)
    return np.asarray(out["y"], np.float32), np.asarray(out["hT"], np.float32)
